# revision 19
# baseline (speedup 1.0000x reference)
"""Trainium2 Bass kernel for CosineAttention:

    out = sigmoid((xn @ xn.T) @ x)   where xn = x / ||x_row||

Key algebraic optimization: reassociate (xn @ xn.T) @ x = xn @ (xn.T @ x).
G = xn.T @ x is [D, D] — the O(N^2 D) similarity matrix is never formed.
Total work drops from ~275 GFLOP to ~34 GFLOP.

Sharding: rows of x across 8 cores. Each core:
  1. loads its [N/8, D] row block, computes row norms + normalized rows
  2. computes partial G_c = xn_c.T @ x_c  (f32 PSUM accumulation)
  3. AllReduce(G) across the 8 cores (2 column chunks, overlapped)
  4. out_c = sigmoid(xn_c @ G)
The host concatenates the 8 row blocks.

Precision (bf16 config): G's diagonal (~256) dwarfs its off-diagonal
entries (~3), so bf16 rounding of G would put ~0.5 absolute error on the
diagonal. We instead compute G' = G - c*I in mm1 (by accumulating
-c * shifted-identity into the PSUM), AllReduce/round G' (small entries,
small absolute error), and add the c*xn term back exactly in f32 before
the sigmoid: z = xn_bf @ G'_bf + c*xn.

Schedule:
  - tiny warmup AllReduce at t~0 absorbs the first-collective
    barrier/launch-skew window under the compute phase
  - mm1 is row-tile-outer so PE starts as soon as tile 0 is loaded
  - G AllReduce is split into two column halves; mm2 on half 0 overlaps
    the half-1 reduce
  - xn.T built by DMA transpose (bf16) or PE transpose (f32r config)
"""

import numpy as np

import concourse.bass as bass  # noqa: F401
import concourse.mybir as mybir
import concourse.tile as tile
from concourse import bacc
from concourse.bass_utils import run_bass_kernel_spmd
from concourse.masks import make_identity

F32 = mybir.dt.float32
F32R = mybir.dt.float32r
BF16 = mybir.dt.bfloat16
F16 = mybir.dt.float16
AFT = mybir.ActivationFunctionType

N, D = 8192, 1024
NCORES = 8
R = N // NCORES  # rows per core
P = 128
RT = R // P      # row tiles per core
KT = D // P      # contraction tiles (mm2) / G row tiles
FD = 512         # matmul moving free dim (one PSUM bank of f32)
NH = D // FD     # column halves
GROUPS = [list(range(NCORES))]
DIAG_C = 256.0   # ~mean of diag(G); exact in bf16


def _emit_body(tc, xb, out, mm_dt, ar_dt, ctx, use_diag=True):
    nc = tc.nc
    xb_t = xb.rearrange("(rt p) d -> rt p d", p=P)
    out_t = out.rearrange("(rt p) d -> rt p d", p=P)
    f32r_mode = mm_dt == F32R
    diag_trick = mm_dt in (BF16, F16) and use_diag

    persist = ctx.enter_context(tc.tile_pool(name="persist", bufs=1))
    load = ctx.enter_context(tc.tile_pool(name="load", bufs=3))
    small = ctx.enter_context(tc.tile_pool(name="small", bufs=1))
    gloc = ctx.enter_context(tc.tile_pool(name="gloc", bufs=3))
    gstage = ctx.enter_context(tc.tile_pool(name="gstage", bufs=3))
    ostage = ctx.enter_context(tc.tile_pool(name="ostage", bufs=3))
    ps = ctx.enter_context(tc.tile_pool(name="ps", bufs=1, space="PSUM"))
    dram = ctx.enter_context(tc.tile_pool(name="dram", bufs=1, space="DRAM"))

    # ---- warmup collective: absorbs first-collective barrier/skew.
    # Input is uninitialized garbage — output is unused; the point is to
    # get this core's first CC doorbell written as early as possible.
    # AllGather has the lowest floor of the collective ops.
    w_in = dram.tile([P, 4], F32, tag="w_in")
    w_out = dram.tile([P * NCORES, 4], F32, tag="w_out")
    nc.gpsimd.collective_compute(
        "AllGather", mybir.AluOpType.bypass, replica_groups=GROUPS,
        ins=[w_in.opt()], outs=[w_out.opt()],
    )

    if f32r_mode:
        ident = persist.tile([P, P], F32, tag="ident")
        make_identity(nc, ident)
    if not f32r_mode:
        identb = persist.tile([P, P], mm_dt, tag="identb")
        make_identity(nc, identb)
    if diag_trick:
        # dsh[s]: [P, FD] bf16, -c * identity placed at columns [s*128,(s+1)*128)
        dsh = []
        for s in range(FD // P):
            t_dsh = persist.tile([P, FD], mm_dt, tag=f"dsh{s}", name=f"dsh{s}")
            nc.vector.memset(t_dsh, 0.0)
            nc.scalar.mul(t_dsh[:, s * P:(s + 1) * P], identb, -DIAG_C / NCORES)
            dsh.append(t_dsh)

    # ---- phase 0: load row block, norms, casts ----
    # Norms are batched: all squares accumulate into columns of one
    # [P, RT] tile, then ONE sqrt + ONE reciprocal — this avoids ACT
    # activation-table thrashing (table swaps cost ~1.4us each). cxn
    # (f32, needs xf) is deferred to the AllReduce window.
    xbr, xnr, xfs = [], [], []
    ss_all = small.tile([P, RT], F32, tag="ss_all")
    for rt in range(RT):
        xf = persist.tile([P, D], F32, tag=f"xf{rt}")
        nc.sync.dma_start(out=xf, in_=xb_t[rt])
        sq = load.tile([P, D], BF16, tag="sq")
        nc.scalar.activation(out=sq, in_=xf, func=AFT.Square,
                             accum_out=ss_all[:, rt:rt + 1])
        t_xbr = persist.tile([P, D], mm_dt, tag=f"xbr{rt}")
        nc.scalar.copy(t_xbr, xf)
        xbr.append(t_xbr)
        xfs.append(xf)
    nrm_all = small.tile([P, RT], F32, tag="nrm_all")
    nc.scalar.sqrt(nrm_all, ss_all)
    rn_all = small.tile([P, RT], F32, tag="rn_all")
    nc.vector.reciprocal(rn_all, nrm_all)
    for rt in range(RT):
        t_xnr = persist.tile([P, D], mm_dt, tag=f"xnr{rt}")
        nc.vector.tensor_scalar_mul(t_xnr, xfs[rt], rn_all[:, rt:rt + 1])
        xnr.append(t_xnr)

    # ---- phase 1: G'_c = xn_c.T @ x_c (- c*I/NCORES), single AllReduce ----
    g_in = dram.tile([D, D], ar_dt, tag="g_in", name="g_in")
    g_out = dram.tile([D, D], ar_dt, tag="g_out", name="g_out")
    g_in_t = g_in.rearrange("(mt p) f -> mt p f", p=P)
    g_out_t = g_out.rearrange("(kt p) f -> kt p f", p=P)
    for nh in range(NH):
        psg = [ps.tile([P, FD], F32, tag=f"acc{mt}", name=f"psg{nh}_{mt}")
               for mt in range(KT)]
        for rt in range(RT):
            for mt in range(KT):
                has_diag = diag_trick and (mt // (FD // P) == nh)
                nc.tensor.matmul(
                    psg[mt],
                    lhsT=xnr[rt][:, mt * P:(mt + 1) * P],
                    rhs=xbr[rt][:, nh * FD:(nh + 1) * FD],
                    start=(rt == 0),
                    stop=(rt == RT - 1) and not has_diag,
                )
        if diag_trick:
            # diag-containing tiles get one extra matmul: += -c/8 * shifted I
            for mt in range(KT):
                if mt // (FD // P) == nh:
                    nc.tensor.matmul(
                        psg[mt], lhsT=identb, rhs=dsh[mt % (FD // P)],
                        start=False, stop=True,
                    )
        for mt in range(KT):
            gl = gloc.tile([P, FD], ar_dt, tag="gloc")
            nc.vector.tensor_copy(out=gl, in_=psg[mt])
            nc.sync.dma_start(out=g_in_t[mt][:, nh * FD:(nh + 1) * FD], in_=gl)
    nc.gpsimd.collective_compute(
        "AllReduce", mybir.AluOpType.add, replica_groups=GROUPS,
        ins=[g_in.opt()], outs=[g_out.opt()],
    )

    # ---- cxn = c*xn in f32, computed during the AllReduce window ----
    cxn = []
    if diag_trick:
        rc_all = small.tile([P, RT], F32, tag="rc_all")
        nc.scalar.mul(rc_all, rn_all, DIAG_C)
        for rt in range(RT):
            t_cxn = persist.tile([P, D], F32, tag=f"cxn{rt}")
            nc.vector.tensor_scalar_mul(t_cxn, xfs[rt], rc_all[:, rt:rt + 1])
            cxn.append(t_cxn)

    # ---- phase 1.5: xnT (DMA transpose for bf16, PE transpose for f32r) ----
    xnT = []
    for kt in range(KT):
        t_xnT = persist.tile([P, D], mm_dt, tag=f"xnT{kt}")
        for rt in range(RT):
            src = xnr[rt][:, kt * P:(kt + 1) * P]
            if f32r_mode:
                tpt = ps.tile([P, P], F32, tag=f"acc{rt % 2}", name=f"tp{kt}_{rt}")
                nc.tensor.transpose(tpt, src.bitcast(F32), ident)
            else:
                tpt = ps.tile([P, P], mm_dt, tag=f"acc{rt % 2}", name=f"tp{kt}_{rt}")
                nc.tensor.transpose(tpt, src, identb)
            nc.vector.tensor_copy(out=t_xnT[:, rt * P:(rt + 1) * P], in_=tpt)
        xnT.append(t_xnT)

    # ---- PE keep-warm ladder across the AllReduce window ----
    # PE sits idle ~50us waiting for the AllReduce; the HAM clock gate
    # re-throttles it to 1.2 GHz after ~3.4us idle, making mm2 run ~2x
    # slow. A PE->DVE->PE dependency ladder of junk transposes keeps the
    # PE active every ~1us. Rung count is sized to stay under the
    # minimum observed AllReduce window so mm2 is never blocked.
    if not f32r_mode:
        wa = persist.tile([P, P], mm_dt, tag="warmA")
        nc.vector.memset(wa, 0.0)
        for w in range(32):
            wp = ps.tile([P, P], mm_dt, tag=f"acc{w % 2}", name=f"warm{w}")
            nc.tensor.transpose(wp, wa, identb)
            nc.vector.tensor_copy(out=wa, in_=wp)

    # ---- phases 3+4: load G back (round for f32r), mm2, sigmoid ----
    gr = []
    for kt in range(KT):
        t_gr = persist.tile([P, D], mm_dt, tag=f"gr{kt}", name=f"gr{kt}")
        if f32r_mode:
            gs = gstage.tile([P, D], F32, tag="gs")
            nc.sync.dma_start(out=gs, in_=g_out_t[kt])
            nc.vector.tensor_copy(out=t_gr, in_=gs)
        else:
            nc.sync.dma_start(out=t_gr, in_=g_out_t[kt])
        gr.append(t_gr)
    for nh in range(NH):
        for mt in range(RT):
            ps_z = ps.tile([P, FD], F32, tag=f"acc{mt}", name=f"psz{nh}_{mt}")
            for kt in range(KT):
                nc.tensor.matmul(
                    ps_z,
                    lhsT=xnT[kt][:, mt * P:(mt + 1) * P],
                    rhs=gr[kt][:, nh * FD:(nh + 1) * FD],
                    start=(kt == 0),
                    stop=(kt == KT - 1),
                )
            if diag_trick:
                nc.vector.tensor_add(
                    ps_z, ps_z, cxn[mt][:, nh * FD:(nh + 1) * FD]
                )
            ob = ostage.tile([P, FD], F32, tag="ob")
            nc.scalar.activation(out=ob, in_=ps_z, func=AFT.Sigmoid)
            nc.sync.dma_start(out=out_t[mt][:, nh * FD:(nh + 1) * FD], in_=ob)


def build(mm_dt=F16, ar_dt=F16, use_diag=True):
    from contextlib import ExitStack

    nc = bacc.Bacc("TRN2", target_bir_lowering=False, debug=False,
                   num_devices=NCORES)
    xb = nc.dram_tensor("xb", [R, D], F32, kind="ExternalInput").ap()
    out = nc.dram_tensor("out", [R, D], F32, kind="ExternalOutput").ap()
    with tile.TileContext(nc) as tc:
        with ExitStack() as ctx:
            _emit_body(tc, xb, out, mm_dt, ar_dt, ctx, use_diag)
    nc.compile()
    return nc


_NC_CACHE = {}


def _get_nc(mm_dt=F16, ar_dt=F16):
    key = (str(mm_dt), str(ar_dt))
    if key not in _NC_CACHE:
        _NC_CACHE[key] = build(mm_dt, ar_dt)
    return _NC_CACHE[key]


def kernel(x: np.ndarray) -> np.ndarray:
    x = np.asarray(x, dtype=np.float32)
    assert x.shape == (N, D), x.shape
    nc = _get_nc()
    in_maps = [{"xb": x[c * R:(c + 1) * R]} for c in range(NCORES)]
    res = run_bass_kernel_spmd(nc, in_maps, list(range(NCORES)))
    return np.concatenate([res.results[c]["out"] for c in range(NCORES)], axis=0)


# revision 20
# speedup vs baseline: 1.0933x; 1.0933x over previous
"""Trainium2 Bass kernel for CosineAttention:

    out = sigmoid((xn @ xn.T) @ x)   where xn = x / ||x_row||

Key algebraic optimization: reassociate (xn @ xn.T) @ x = xn @ (xn.T @ x).
G = xn.T @ x is [D, D] — the O(N^2 D) similarity matrix is never formed.
Total work drops from ~275 GFLOP to ~34 GFLOP.

Sharding: rows of x across 8 cores. Each core:
  1. loads its [N/8, D] row block, computes row norms + normalized rows
  2. computes partial G_c = xn_c.T @ x_c  (f32 PSUM accumulation)
  3. AllReduce(G) across the 8 cores (2 column chunks, overlapped)
  4. out_c = sigmoid(xn_c @ G)
The host concatenates the 8 row blocks.

Precision (bf16 config): G's diagonal (~256) dwarfs its off-diagonal
entries (~3), so bf16 rounding of G would put ~0.5 absolute error on the
diagonal. We instead compute G' = G - c*I in mm1 (by accumulating
-c * shifted-identity into the PSUM), AllReduce/round G' (small entries,
small absolute error), and add the c*xn term back exactly in f32 before
the sigmoid: z = xn_bf @ G'_bf + c*xn.

Schedule:
  - tiny warmup AllReduce at t~0 absorbs the first-collective
    barrier/launch-skew window under the compute phase
  - mm1 is row-tile-outer so PE starts as soon as tile 0 is loaded
  - G AllReduce is split into two column halves; mm2 on half 0 overlaps
    the half-1 reduce
  - xn.T built by DMA transpose (bf16) or PE transpose (f32r config)
"""

import numpy as np

import concourse.bass as bass  # noqa: F401
import concourse.mybir as mybir
import concourse.tile as tile
from concourse import bacc
from concourse.bass_utils import run_bass_kernel_spmd
from concourse.masks import make_identity

F32 = mybir.dt.float32
F32R = mybir.dt.float32r
BF16 = mybir.dt.bfloat16
F16 = mybir.dt.float16
AFT = mybir.ActivationFunctionType

N, D = 8192, 1024
NCORES = 8
R = N // NCORES  # rows per core
P = 128
RT = R // P      # row tiles per core
KT = D // P      # contraction tiles (mm2) / G row tiles
FD = 512         # matmul moving free dim (one PSUM bank of f32)
NH = D // FD     # column halves
GROUPS = [list(range(NCORES))]
DIAG_C = 256.0   # ~mean of diag(G); exact in bf16


def _emit_body(tc, xb, out, mm_dt, ar_dt, ctx, use_diag=True):
    nc = tc.nc
    xb_t = xb.rearrange("(rt p) d -> rt p d", p=P)
    out_t = out.rearrange("(rt p) d -> rt p d", p=P)
    f32r_mode = mm_dt == F32R
    diag_trick = mm_dt in (BF16, F16) and use_diag

    persist = ctx.enter_context(tc.tile_pool(name="persist", bufs=1))
    load = ctx.enter_context(tc.tile_pool(name="load", bufs=3))
    small = ctx.enter_context(tc.tile_pool(name="small", bufs=1))
    gloc = ctx.enter_context(tc.tile_pool(name="gloc", bufs=3))
    gstage = ctx.enter_context(tc.tile_pool(name="gstage", bufs=3))
    ostage = ctx.enter_context(tc.tile_pool(name="ostage", bufs=3))
    ps = ctx.enter_context(tc.tile_pool(name="ps", bufs=1, space="PSUM"))
    dram = ctx.enter_context(tc.tile_pool(name="dram", bufs=1, space="DRAM"))

    # ---- warmup collective: absorbs first-collective barrier/skew.
    # Input is uninitialized garbage — output is unused; the point is to
    # get this core's first CC doorbell written as early as possible.
    # AllGather has the lowest floor of the collective ops.
    w_in = dram.tile([P, 4], F32, tag="w_in")
    w_out = dram.tile([P * NCORES, 4], F32, tag="w_out")
    nc.gpsimd.collective_compute(
        "AllGather", mybir.AluOpType.bypass, replica_groups=GROUPS,
        ins=[w_in.opt()], outs=[w_out.opt()],
    )

    if f32r_mode:
        ident = persist.tile([P, P], F32, tag="ident")
        make_identity(nc, ident)
    if not f32r_mode:
        identb = persist.tile([P, P], mm_dt, tag="identb")
        make_identity(nc, identb)
    if diag_trick:
        # dsh[s]: [P, FD] bf16, -c * identity placed at columns [s*128,(s+1)*128)
        dsh = []
        for s in range(FD // P):
            t_dsh = persist.tile([P, FD], mm_dt, tag=f"dsh{s}", name=f"dsh{s}")
            nc.vector.memset(t_dsh, 0.0)
            nc.scalar.mul(t_dsh[:, s * P:(s + 1) * P], identb, -DIAG_C / NCORES)
            dsh.append(t_dsh)

    # ---- phase 0: load row block, norms, casts ----
    # Norms are batched: all squares accumulate into columns of one
    # [P, RT] tile, then ONE sqrt + ONE reciprocal — this avoids ACT
    # activation-table thrashing (table swaps cost ~1.4us each). cxn
    # (f32, needs xf) is deferred to the AllReduce window.
    # All ACT Squares run back-to-back (one activation table), then one
    # batched sqrt; the x casts go on DVE so ACT never swaps tables.
    xbr, xnr, xfs = [], [], []
    ss_all = small.tile([P, RT], F32, tag="ss_all")
    for rt in range(RT):
        xf = persist.tile([P, D], F32, tag=f"xf{rt}")
        nc.sync.dma_start(out=xf, in_=xb_t[rt])
        sq = load.tile([P, D], BF16, tag="sq")
        nc.scalar.activation(out=sq, in_=xf, func=AFT.Square,
                             accum_out=ss_all[:, rt:rt + 1])
        t_xbr = persist.tile([P, D], mm_dt, tag=f"xbr{rt}")
        nc.vector.tensor_copy(out=t_xbr, in_=xf)
        xbr.append(t_xbr)
        xfs.append(xf)
    nrm_all = small.tile([P, RT], F32, tag="nrm_all")
    nc.scalar.sqrt(nrm_all, ss_all)
    rn_all = small.tile([P, RT], F32, tag="rn_all")
    nc.vector.reciprocal(rn_all, nrm_all)
    for rt in range(RT):
        t_xnr = persist.tile([P, D], mm_dt, tag=f"xnr{rt}")
        nc.vector.tensor_scalar_mul(t_xnr, xfs[rt], rn_all[:, rt:rt + 1])
        xnr.append(t_xnr)

    # ---- phase 1: G'_c = xn_c.T @ x_c (- c*I/NCORES), single AllReduce ----
    g_in = dram.tile([D, D], ar_dt, tag="g_in", name="g_in")
    g_out = dram.tile([D, D], ar_dt, tag="g_out", name="g_out")
    g_in_t = g_in.rearrange("(mt p) f -> mt p f", p=P)
    g_out_t = g_out.rearrange("(kt p) f -> kt p f", p=P)
    for nh in range(NH):
        psg = [ps.tile([P, FD], F32, tag=f"acc{mt}", name=f"psg{nh}_{mt}")
               for mt in range(KT)]
        for rt in range(RT):
            for mt in range(KT):
                has_diag = diag_trick and (mt // (FD // P) == nh)
                nc.tensor.matmul(
                    psg[mt],
                    lhsT=xnr[rt][:, mt * P:(mt + 1) * P],
                    rhs=xbr[rt][:, nh * FD:(nh + 1) * FD],
                    start=(rt == 0),
                    stop=(rt == RT - 1) and not has_diag,
                )
        if diag_trick:
            # diag-containing tiles get one extra matmul: += -c/8 * shifted I
            for mt in range(KT):
                if mt // (FD // P) == nh:
                    nc.tensor.matmul(
                        psg[mt], lhsT=identb, rhs=dsh[mt % (FD // P)],
                        start=False, stop=True,
                    )
        for mt in range(KT):
            gl = gloc.tile([P, FD], ar_dt, tag="gloc")
            nc.vector.tensor_copy(out=gl, in_=psg[mt])
            nc.sync.dma_start(out=g_in_t[mt][:, nh * FD:(nh + 1) * FD], in_=gl)
    nc.gpsimd.collective_compute(
        "AllReduce", mybir.AluOpType.add, replica_groups=GROUPS,
        ins=[g_in.opt()], outs=[g_out.opt()],
    )

    # ---- cxn = c*xn in f32, computed during the AllReduce window ----
    cxn = []
    if diag_trick:
        rc_all = small.tile([P, RT], F32, tag="rc_all")
        nc.scalar.mul(rc_all, rn_all, DIAG_C)
        for rt in range(RT):
            t_cxn = persist.tile([P, D], F32, tag=f"cxn{rt}")
            nc.vector.tensor_scalar_mul(t_cxn, xfs[rt], rc_all[:, rt:rt + 1])
            cxn.append(t_cxn)

    # ---- phase 1.5: xnT (DMA transpose for bf16, PE transpose for f32r) ----
    xnT = []
    for kt in range(KT):
        t_xnT = persist.tile([P, D], mm_dt, tag=f"xnT{kt}")
        for rt in range(RT):
            src = xnr[rt][:, kt * P:(kt + 1) * P]
            if f32r_mode:
                tpt = ps.tile([P, P], F32, tag=f"acc{rt % 2}", name=f"tp{kt}_{rt}")
                nc.tensor.transpose(tpt, src.bitcast(F32), ident)
            else:
                tpt = ps.tile([P, P], mm_dt, tag=f"acc{rt % 2}", name=f"tp{kt}_{rt}")
                nc.tensor.transpose(tpt, src, identb)
            nc.vector.tensor_copy(out=t_xnT[:, rt * P:(rt + 1) * P], in_=tpt)
        xnT.append(t_xnT)

    # ---- PE keep-warm ladder across the AllReduce window ----
    # PE sits idle ~50us waiting for the AllReduce; the HAM clock gate
    # re-throttles it to 1.2 GHz after ~3.4us idle, making mm2 run ~2x
    # slow. A PE->DVE->PE dependency ladder of junk transposes keeps the
    # PE active every ~1us. Rung count is sized to stay under the
    # minimum observed AllReduce window so mm2 is never blocked.
    if not f32r_mode:
        wa = persist.tile([P, P], mm_dt, tag="warmA")
        nc.vector.memset(wa, 0.0)
        for w in range(32):
            wp = ps.tile([P, P], mm_dt, tag=f"acc{w % 2}", name=f"warm{w}")
            nc.tensor.transpose(wp, wa, identb)
            nc.vector.tensor_copy(out=wa, in_=wp)

    # ---- phases 3+4: load G back (round for f32r), mm2, sigmoid ----
    gr = []
    for kt in range(KT):
        t_gr = persist.tile([P, D], mm_dt, tag=f"gr{kt}", name=f"gr{kt}")
        if f32r_mode:
            gs = gstage.tile([P, D], F32, tag="gs")
            nc.sync.dma_start(out=gs, in_=g_out_t[kt])
            nc.vector.tensor_copy(out=t_gr, in_=gs)
        else:
            nc.sync.dma_start(out=t_gr, in_=g_out_t[kt])
        gr.append(t_gr)
    for nh in range(NH):
        for mt in range(RT):
            ps_z = ps.tile([P, FD], F32, tag=f"acc{mt}", name=f"psz{nh}_{mt}")
            for kt in range(KT):
                nc.tensor.matmul(
                    ps_z,
                    lhsT=xnT[kt][:, mt * P:(mt + 1) * P],
                    rhs=gr[kt][:, nh * FD:(nh + 1) * FD],
                    start=(kt == 0),
                    stop=(kt == KT - 1),
                )
            if diag_trick:
                nc.vector.tensor_add(
                    ps_z, ps_z, cxn[mt][:, nh * FD:(nh + 1) * FD]
                )
            ob = ostage.tile([P, FD], F32, tag="ob")
            nc.scalar.activation(out=ob, in_=ps_z, func=AFT.Sigmoid)
            nc.sync.dma_start(out=out_t[mt][:, nh * FD:(nh + 1) * FD], in_=ob)


def build(mm_dt=F16, ar_dt=F16, use_diag=True):
    from contextlib import ExitStack

    nc = bacc.Bacc("TRN2", target_bir_lowering=False, debug=False,
                   num_devices=NCORES)
    xb = nc.dram_tensor("xb", [R, D], F32, kind="ExternalInput").ap()
    out = nc.dram_tensor("out", [R, D], F32, kind="ExternalOutput").ap()
    with tile.TileContext(nc) as tc:
        with ExitStack() as ctx:
            _emit_body(tc, xb, out, mm_dt, ar_dt, ctx, use_diag)
    nc.compile()
    return nc


_NC_CACHE = {}


def _get_nc(mm_dt=F16, ar_dt=F16):
    key = (str(mm_dt), str(ar_dt))
    if key not in _NC_CACHE:
        _NC_CACHE[key] = build(mm_dt, ar_dt)
    return _NC_CACHE[key]


def kernel(x: np.ndarray) -> np.ndarray:
    x = np.asarray(x, dtype=np.float32)
    assert x.shape == (N, D), x.shape
    nc = _get_nc()
    in_maps = [{"xb": x[c * R:(c + 1) * R]} for c in range(NCORES)]
    res = run_bass_kernel_spmd(nc, in_maps, list(range(NCORES)))
    return np.concatenate([res.results[c]["out"] for c in range(NCORES)], axis=0)
